# revision 4
# baseline (speedup 1.0000x reference)
"""MoE layer (top-2 of 8 experts, SwiGLU) on 8 Trainium2 NeuronCores.

Strategy (expert-parallel, matching the sharding hint):
  - Host computes the router (gate logits -> top-2 -> softmax) in fp32 numpy,
    exactly mirroring the reference math. This is the "token dispatch" step:
    tokens are gathered per expert on the host (the all-to-all), each core
    gets one expert's weights plus that expert's tokens.
  - Each core runs a dense SwiGLU MLP over its gathered token batch:
        h = silu(x @ w_gate.T) * (x @ w1.T);  y = h @ w2.T
    Matmuls run in bf16 with fp32 PSUM accumulation.
  - Host multiplies each expert's outputs by the combine weights and
    scatter-adds back into token order (the "combine" step).

Layouts are pre-swizzled on host so every DMA is a simple strided copy:
  xt  [128, 8, C]    : x gathered+transposed, d = ko*128 + p
  w1t [128, 8, 4096] : w1.T   (d on partitions)
  wgt [128, 8, 4096] : w_gate.T
  w2t [128, 32, 1024]: w2.T   (d_ff on partitions)
  y   [C, 1024]      : fp32 output (token-major)
"""

import numpy as np
import ml_dtypes

import concourse.bass as bass
import concourse.mybir as mybir
import concourse.tile as tile
from concourse.bass_utils import run_bass_kernel_spmd

# ---------------------------------------------------------------------------
# Workaround for this walrus build: TPB instructions have a single hardware
# wait slot and this walrus refuses any instruction carrying more than one
# sem wait ("Too many sync wait commands"). Post-pass: for every instruction
# with k>1 waits, hoist k-1 waits onto single-wait NOPs on the same engine
# immediately before it. Program-order semantics are identical (the engine
# blocks on each wait in turn before issuing the instruction).
# ---------------------------------------------------------------------------

_ws_counter = [0]


def _split_multi_waits(nc: bass.Bass) -> int:
    n_split = 0
    for f in nc.m.functions:
        for bb in f.blocks:
            new_insts = []
            for inst in bb.instructions:
                si = inst.sync_info
                if si is not None and si.on_wait and len(si.on_wait) > 1:
                    waits = list(si.on_wait)
                    for w in waits[:-1]:
                        _ws_counter[0] += 1
                        n_split += 1
                        new_insts.append(
                            mybir.InstNoOp(
                                name=f"waitsplit-{_ws_counter[0]}",
                                opcode="NoOp",
                                engine=inst.engine,
                                sync_info=mybir.SyncInfo(
                                    on_wait=[w], on_update=[]
                                ),
                                bass_nofuse=True,
                                text_hint="waitsplit",
                            )
                        )
                    si.on_wait = [waits[-1]]
                new_insts.append(inst)
            bb.instructions[:] = new_insts
    return n_split

# ---------------------------------------------------------------------------

D = 1024
DFF = 4096
N_EXPERTS = 8
TOP_K = 2
N_CORES = 8
TB = 512          # token block processed per outer iteration
KD = D // 128     # 8 contraction tiles over d
NF = DFF // 128   # 32 tiles over d_ff

BF16 = mybir.dt.bfloat16
F32 = mybir.dt.float32
NP_BF16 = ml_dtypes.bfloat16

_NC_CACHE: dict[int, bass.Bass] = {}


def _build_kernel(C: int) -> bass.Bass:
    """Dense SwiGLU MLP over C tokens (token batch padded to TB multiple)."""
    assert C % TB == 0
    nb = C // TB

    nc = bass.Bass()
    xt = nc.dram_tensor("xt", [128, KD, C], BF16, kind="ExternalInput")
    w1t = nc.dram_tensor("w1t", [128, KD, DFF], BF16, kind="ExternalInput")
    wgt = nc.dram_tensor("wgt", [128, KD, DFF], BF16, kind="ExternalInput")
    w2t = nc.dram_tensor("w2t", [128, NF, D], BF16, kind="ExternalInput")
    y = nc.dram_tensor("y", [C, D], F32, kind="ExternalOutput")

    silu = mybir.ActivationFunctionType.Silu

    with tile.TileContext(nc) as tc:
        with (
            tc.tile_pool(name="wres", bufs=1) as wres,
            tc.tile_pool(name="wg", bufs=3) as wgpool,
            tc.tile_pool(name="xt", bufs=2) as xtpool,
            tc.tile_pool(name="hg", bufs=3) as hgpool,
            tc.tile_pool(name="h", bufs=40) as hpool,
            tc.tile_pool(name="w2", bufs=4) as w2pool,
            tc.tile_pool(name="yo", bufs=4) as ypool,
            tc.tile_pool(name="ps1", bufs=2, space="PSUM") as psum1,
            tc.tile_pool(name="ps2", bufs=4, space="PSUM") as psum2,
        ):
            w1_sb = wres.tile([128, KD, DFF], BF16)
            nc.sync.dma_start(w1_sb[:], w1t[:])

            for b in range(nb):
                xt_sb = xtpool.tile([128, KD, TB], BF16)
                nc.sync.dma_start(xt_sb[:], xt[:, :, b * TB:(b + 1) * TB])

                h_tiles = []
                for dfc in range(NF // 4):
                    wg_ch = wgpool.tile([128, KD, 512], BF16)
                    nc.sync.dma_start(
                        wg_ch[:], wgt[:, :, dfc * 512:(dfc + 1) * 512]
                    )
                    for j in range(4):
                        df = dfc * 4 + j
                        psg = psum1.tile([128, TB], F32, tag="psg")
                        for d in range(KD):
                            nc.tensor.matmul(
                                psg[:],
                                wg_ch[:, d, j * 128:(j + 1) * 128],
                                xt_sb[:, d, :],
                                start=(d == 0),
                                stop=(d == KD - 1),
                            )
                        ps1t = psum1.tile([128, TB], F32, tag="ps1t")
                        for d in range(KD):
                            nc.tensor.matmul(
                                ps1t[:],
                                w1_sb[:, d, df * 128:(df + 1) * 128],
                                xt_sb[:, d, :],
                                start=(d == 0),
                                stop=(d == KD - 1),
                            )
                        hg = hgpool.tile([128, TB], BF16)
                        nc.scalar.activation(hg[:], psg[:], silu)
                        h = hpool.tile([128, TB], BF16, tag="h")
                        nc.vector.tensor_mul(h[:], hg[:], ps1t[:])
                        h_tiles.append(h)

                for half in range(2):
                    psys = [
                        psum2.tile([128, 512], F32, tag="psy", name=f"psy{m}")
                        for m in range(4)
                    ]
                    for df in range(NF):
                        w2_ch = w2pool.tile([128, 512], BF16)
                        nc.sync.dma_start(
                            w2_ch[:], w2t[:, df, half * 512:(half + 1) * 512]
                        )
                        for m in range(4):
                            nc.tensor.matmul(
                                psys[m][:],
                                h_tiles[df][:, m * 128:(m + 1) * 128],
                                w2_ch[:],
                                start=(df == 0),
                                stop=(df == NF - 1),
                            )
                    for m in range(4):
                        y_sb = ypool.tile([128, 512], F32)
                        nc.vector.tensor_copy(y_sb[:], psys[m][:])
                        nc.sync.dma_start(
                            y[
                                b * TB + m * 128: b * TB + (m + 1) * 128,
                                half * 512:(half + 1) * 512,
                            ],
                            y_sb[:],
                        )
    _split_multi_waits(nc)
    return nc


def _swizzle_k(a: np.ndarray) -> np.ndarray:
    """[K, F] -> [128, K//128, F] with K = ko*128 + p on partitions."""
    k, f = a.shape
    return np.ascontiguousarray(
        a.reshape(k // 128, 128, f).transpose(1, 0, 2)
    )


def kernel(x, gate_w, w1, w_gate, w2):
    b, t, d = x.shape
    xf = np.ascontiguousarray(x.reshape(-1, d)).astype(np.float32)
    n_tok = xf.shape[0]

    # --- Router (host, fp32, mirrors reference math) ---
    logits = xf @ gate_w.T.astype(np.float32)                  # [N, E]
    top_idx = np.argsort(-logits, axis=1, kind="stable")[:, :TOP_K]  # [N, K]
    top_vals = np.take_along_axis(logits, top_idx, axis=1)
    m = top_vals.max(axis=1, keepdims=True)
    ex = np.exp(top_vals - m)
    top_w = ex / ex.sum(axis=1, keepdims=True)                 # [N, K]

    pair_expert = top_idx.reshape(-1)                          # [N*K]
    pair_w = top_w.reshape(-1).astype(np.float32)
    order = np.argsort(pair_expert, kind="stable")
    counts = np.bincount(pair_expert, minlength=N_EXPERTS)
    starts = np.concatenate([[0], np.cumsum(counts)])

    C = max(TB, int(-(-counts.max() // TB)) * TB)

    # --- Build per-core inputs (dispatch) ---
    in_maps = []
    sels = []
    for e in range(N_EXPERTS):
        sel = order[starts[e]:starts[e + 1]]
        sels.append(sel)
        tok = sel // TOP_K
        xt_full = np.zeros((D, C), dtype=np.float32)
        xt_full[:, : len(tok)] = xf[tok].T
        in_maps.append(
            {
                "xt": _swizzle_k(xt_full).astype(NP_BF16),
                "w1t": _swizzle_k(
                    np.ascontiguousarray(w1[e].T).astype(np.float32)
                ).astype(NP_BF16),
                "wgt": _swizzle_k(
                    np.ascontiguousarray(w_gate[e].T).astype(np.float32)
                ).astype(NP_BF16),
                "w2t": _swizzle_k(
                    np.ascontiguousarray(w2[e].T).astype(np.float32)
                ).astype(NP_BF16),
            }
        )

    if C not in _NC_CACHE:
        _NC_CACHE[C] = _build_kernel(C)
    nc = _NC_CACHE[C]

    res = run_bass_kernel_spmd(nc, in_maps, core_ids=list(range(N_CORES)))

    # --- Combine (host): weight by router prob, scatter-add to tokens ---
    contrib = np.zeros((n_tok * TOP_K, D), dtype=np.float32)
    for e in range(N_EXPERTS):
        sel = sels[e]
        y_e = res.results[e]["y"][: len(sel)]
        contrib[sel] = y_e * pair_w[sel][:, None]
    out = contrib.reshape(n_tok, TOP_K, D).sum(axis=1)
    return out.reshape(b, t, d).astype(x.dtype)


# revision 9
# speedup vs baseline: 1.1240x; 1.1240x over previous
"""MoE layer (top-2 of 8 experts, SwiGLU) on 8 Trainium2 NeuronCores.

Strategy (expert-parallel, matching the sharding hint):
  - Host computes the router (gate logits -> top-2 -> softmax) in fp32 numpy,
    exactly mirroring the reference math. This is the "token dispatch" step:
    tokens are gathered per expert on the host (the all-to-all), each core
    gets one expert's weights plus that expert's tokens.
  - Each core runs a dense SwiGLU MLP over its gathered token batch:
        h = silu(x @ w_gate.T) * (x @ w1.T);  y = h @ w2.T
    Matmuls run in bf16 with fp32 PSUM accumulation.
  - Host multiplies each expert's outputs by the combine weights and
    scatter-adds back into token order (the "combine" step).

Layouts are pre-swizzled on host so every DMA is a simple strided copy:
  xt  [128, 8, C]    : x gathered+transposed, d = ko*128 + p
  w1t [128, 8, 4096] : w1.T   (d on partitions)
  wgt [128, 8, 4096] : w_gate.T
  w2t [128, 32, 1024]: w2.T   (d_ff on partitions)
  y   [C, 1024]      : fp32 output (token-major)
"""

import numpy as np
import ml_dtypes

import concourse.bass as bass
import concourse.mybir as mybir
import concourse.tile as tile
from concourse.bass_utils import run_bass_kernel_spmd

# ---------------------------------------------------------------------------
# Workaround for this walrus build: TPB instructions have a single hardware
# wait slot and this walrus refuses any instruction carrying more than one
# sem wait ("Too many sync wait commands"). Post-pass: for every instruction
# with k>1 waits, hoist k-1 waits onto single-wait NOPs on the same engine
# immediately before it. Program-order semantics are identical (the engine
# blocks on each wait in turn before issuing the instruction).
# ---------------------------------------------------------------------------

_ws_counter = [0]


def _split_multi_waits(nc: bass.Bass) -> int:
    n_split = 0
    for f in nc.m.functions:
        for bb in f.blocks:
            new_insts = []
            for inst in bb.instructions:
                si = inst.sync_info
                if si is not None and si.on_wait and len(si.on_wait) > 1:
                    waits = list(si.on_wait)
                    for w in waits[:-1]:
                        _ws_counter[0] += 1
                        n_split += 1
                        new_insts.append(
                            mybir.InstNoOp(
                                name=f"waitsplit-{_ws_counter[0]}",
                                opcode="NoOp",
                                engine=inst.engine,
                                sync_info=mybir.SyncInfo(
                                    on_wait=[w], on_update=[]
                                ),
                                bass_nofuse=True,
                                text_hint="waitsplit",
                            )
                        )
                    si.on_wait = [waits[-1]]
                new_insts.append(inst)
            bb.instructions[:] = new_insts
    return n_split

# ---------------------------------------------------------------------------

D = 1024
DFF = 4096
N_EXPERTS = 8
TOP_K = 2
N_CORES = 8
TB = 512          # token block processed per outer iteration
KD = D // 128     # 8 contraction tiles over d
NF = DFF // 128   # 32 tiles over d_ff

BF16 = mybir.dt.bfloat16
F32 = mybir.dt.float32
NP_BF16 = ml_dtypes.bfloat16

_NC_CACHE: dict[int, bass.Bass] = {}


def _build_kernel(C: int) -> bass.Bass:
    """Dense SwiGLU MLP over C tokens (C a multiple of 128; blocks of 512
    plus one smaller tail block)."""
    assert C % 128 == 0
    blocks = [TB] * (C // TB)
    if C % TB:
        blocks.append(C % TB)

    nc = bass.Bass()
    xt = nc.dram_tensor("xt", [128, KD, C], BF16, kind="ExternalInput")
    w1t = nc.dram_tensor("w1t", [128, KD, DFF], BF16, kind="ExternalInput")
    wgt = nc.dram_tensor("wgt", [128, KD, DFF], BF16, kind="ExternalInput")
    w2t = nc.dram_tensor("w2t", [128, NF, D], BF16, kind="ExternalInput")
    y = nc.dram_tensor("y", [C, D], F32, kind="ExternalOutput")

    silu = mybir.ActivationFunctionType.Silu

    with tile.TileContext(nc) as tc:
        with (
            tc.tile_pool(name="wres", bufs=1) as wres,
            tc.tile_pool(name="wg", bufs=3) as wgpool,
            tc.tile_pool(name="xt", bufs=2) as xtpool,
            tc.tile_pool(name="hg", bufs=3) as hgpool,
            tc.tile_pool(name="h", bufs=40) as hpool,
            tc.tile_pool(name="w2", bufs=4) as w2pool,
            tc.tile_pool(name="yo", bufs=4) as ypool,
            tc.tile_pool(name="ps1", bufs=2, space="PSUM") as psum1,
            tc.tile_pool(name="ps2", bufs=4, space="PSUM") as psum2,
        ):
            w1_sb = wres.tile([128, KD, DFF], BF16)
            nc.sync.dma_start(w1_sb[:], w1t[:])

            tok0 = 0
            for b, tb in enumerate(blocks):
                xt_sb = xtpool.tile([128, KD, tb], BF16, tag="xt")
                nc.sync.dma_start(xt_sb[:], xt[:, :, tok0:tok0 + tb])

                h_tiles = []
                for dfc in range(NF // 4):
                    wg_ch = wgpool.tile([128, KD, 512], BF16)
                    nc.sync.dma_start(
                        wg_ch[:], wgt[:, :, dfc * 512:(dfc + 1) * 512]
                    )
                    for j in range(4):
                        df = dfc * 4 + j
                        psg = psum1.tile([128, tb], F32, tag="psg")
                        for d in range(KD):
                            nc.tensor.matmul(
                                psg[:],
                                wg_ch[:, d, j * 128:(j + 1) * 128],
                                xt_sb[:, d, :],
                                start=(d == 0),
                                stop=(d == KD - 1),
                            )
                        ps1t = psum1.tile([128, tb], F32, tag="ps1t")
                        for d in range(KD):
                            nc.tensor.matmul(
                                ps1t[:],
                                w1_sb[:, d, df * 128:(df + 1) * 128],
                                xt_sb[:, d, :],
                                start=(d == 0),
                                stop=(d == KD - 1),
                            )
                        hg = hgpool.tile([128, tb], BF16, tag="hg")
                        nc.scalar.activation(hg[:], psg[:], silu)
                        h = hpool.tile([128, tb], BF16, tag="h")
                        nc.vector.tensor_mul(h[:], hg[:], ps1t[:])
                        h_tiles.append(h)

                n_m = tb // 128
                for half in range(2):
                    psys = [
                        psum2.tile([128, 512], F32, tag="psy", name=f"psy{m}")
                        for m in range(n_m)
                    ]
                    for df in range(NF):
                        w2_ch = w2pool.tile([128, 512], BF16, tag="w2c")
                        nc.sync.dma_start(
                            w2_ch[:], w2t[:, df, half * 512:(half + 1) * 512]
                        )
                        for m in range(n_m):
                            nc.tensor.matmul(
                                psys[m][:],
                                h_tiles[df][:, m * 128:(m + 1) * 128],
                                w2_ch[:],
                                start=(df == 0),
                                stop=(df == NF - 1),
                            )
                    for m in range(n_m):
                        y_sb = ypool.tile([128, 512], F32, tag="ysb")
                        nc.vector.tensor_copy(y_sb[:], psys[m][:])
                        nc.sync.dma_start(
                            y[
                                tok0 + m * 128: tok0 + (m + 1) * 128,
                                half * 512:(half + 1) * 512,
                            ],
                            y_sb[:],
                        )
                tok0 += tb
    _split_multi_waits(nc)
    return nc


def _swizzle_k(a: np.ndarray) -> np.ndarray:
    """[K, F] -> [128, K//128, F] with K = ko*128 + p on partitions."""
    k, f = a.shape
    return np.ascontiguousarray(
        a.reshape(k // 128, 128, f).transpose(1, 0, 2)
    )


def kernel(x, gate_w, w1, w_gate, w2):
    b, t, d = x.shape
    xf = np.ascontiguousarray(x.reshape(-1, d)).astype(np.float32)
    n_tok = xf.shape[0]

    # --- Router (host, fp32, mirrors reference math) ---
    logits = xf @ gate_w.T.astype(np.float32)                  # [N, E]
    top_idx = np.argsort(-logits, axis=1, kind="stable")[:, :TOP_K]  # [N, K]
    top_vals = np.take_along_axis(logits, top_idx, axis=1)
    m = top_vals.max(axis=1, keepdims=True)
    ex = np.exp(top_vals - m)
    top_w = ex / ex.sum(axis=1, keepdims=True)                 # [N, K]

    pair_expert = top_idx.reshape(-1)                          # [N*K]
    pair_w = top_w.reshape(-1).astype(np.float32)
    order = np.argsort(pair_expert, kind="stable")
    counts = np.bincount(pair_expert, minlength=N_EXPERTS)
    starts = np.concatenate([[0], np.cumsum(counts)])

    C = max(128, int(-(-int(counts.max()) // 128)) * 128)

    # --- Build per-core inputs (dispatch) ---
    in_maps = []
    sels = []
    for e in range(N_EXPERTS):
        sel = order[starts[e]:starts[e + 1]]
        sels.append(sel)
        tok = sel // TOP_K
        xt_full = np.zeros((D, C), dtype=np.float32)
        xt_full[:, : len(tok)] = xf[tok].T
        in_maps.append(
            {
                "xt": _swizzle_k(xt_full).astype(NP_BF16),
                "w1t": _swizzle_k(
                    np.ascontiguousarray(w1[e].T).astype(np.float32)
                ).astype(NP_BF16),
                "wgt": _swizzle_k(
                    np.ascontiguousarray(w_gate[e].T).astype(np.float32)
                ).astype(NP_BF16),
                "w2t": _swizzle_k(
                    np.ascontiguousarray(w2[e].T).astype(np.float32)
                ).astype(NP_BF16),
            }
        )

    if C not in _NC_CACHE:
        _NC_CACHE[C] = _build_kernel(C)
    nc = _NC_CACHE[C]

    res = run_bass_kernel_spmd(nc, in_maps, core_ids=list(range(N_CORES)))

    # --- Combine (host): weight by router prob, scatter-add to tokens ---
    contrib = np.zeros((n_tok * TOP_K, D), dtype=np.float32)
    for e in range(N_EXPERTS):
        sel = sels[e]
        y_e = res.results[e]["y"][: len(sel)]
        contrib[sel] = y_e * pair_w[sel][:, None]
    out = contrib.reshape(n_tok, TOP_K, D).sum(axis=1)
    return out.reshape(b, t, d).astype(x.dtype)
